# revision 32
# baseline (speedup 1.0000x reference)
"""Trainium2 Bass kernel for DeformableTinyImageNet (32x3x64x64 -> 32x200).

Sharding: data-parallel, 32 images -> 8 cores x 4 images; all weights
replicated (tiny vs activations).

Deformable conv is computed exactly as a *modulated convolution*: for
offsets |d| < R, the bilinear sample at (h+ki+dy, w+kj+dx) equals
    sum_{da,db, |da|,|db|<=R} tri(dy-da)*tri(dx-db) * x[c, h+ki+da, w+kj+db]
with tri(t) = max(0, 1-|t|)  (tri has unit support, so the window covers
all corners).  Window radii are validated on the host against the exact
offset fields (offsets are deterministic functions of the inputs).

Layer 1 (Cin=3): partitions pack (img, tap, channel) = 4*(9*3 -> 32) with
the tap shift pre-baked into each replicated row of a padded input copy,
so tri fields are built directly in replicated row space.
Layer 2 (Cin=64): partitions pack (2 images x 64 channels); per-tap tri
products are built in position space [18, HW] and broadcast to the 128
modulate rows with partition-stride-0 DMAs.
"""

import numpy as np
from contextlib import ExitStack

B, C1, H, W = 32, 3, 64, 64
HW = H * W
NCORES = 8
BPC = B // NCORES
C2 = 64
C3 = 128
K2 = 9
EPS = 1e-5

PAD1 = 3
WP1, HP1 = W + 2 * PAD1, H + 2 * PAD1        # 70
PAD2 = 2
WP2, HP2 = W + 2 * PAD2, H + 2 * PAD2        # 68

NCHUNK = 8
CH = HW // NCHUNK                             # 512

_taps = [(ki, kj) for ki in (-1, 0, 1) for kj in (-1, 0, 1)]

# test.py hooks
_SKIP = set()
_TRACE = False
_LAST_RESULTS = None


# --------------------------------------------------------------------------
# host-side numpy helpers (window validation only; all NN math runs on device)
# --------------------------------------------------------------------------

def _conv3x3_np(x, w, b):
    Bn, C, Hh, Ww = x.shape
    xp = np.pad(x, ((0, 0), (0, 0), (1, 1), (1, 1)))
    out = np.zeros((Bn, w.shape[0], Hh, Ww), np.float32)
    for ki in range(3):
        for kj in range(3):
            out += np.einsum("oc,bchw->bohw", w[:, :, ki, kj],
                             xp[:, :, ki:ki + Hh, kj:kj + Ww])
    return out + b[None, :, None, None]


def _deform_np(x, off, w, b):
    Bn, Cin, Hh, Ww = x.shape
    o = off.reshape(Bn, K2, 2, Hh, Ww)
    dy, dx = o[:, :, 0], o[:, :, 1]
    ki = (np.arange(3).repeat(3) - 1).astype(np.float32)
    kj = (np.tile(np.arange(3), 3) - 1).astype(np.float32)
    py = np.arange(Hh, dtype=np.float32)[None, None, :, None] + ki[None, :, None, None] + dy
    px = np.arange(Ww, dtype=np.float32)[None, None, None, :] + kj[None, :, None, None] + dx
    y0, x0 = np.floor(py), np.floor(px)
    wy, wx = py - y0, px - x0
    flat = x.reshape(Bn, Cin, Hh * Ww)

    def g(yc, xc):
        valid = ((yc >= 0) & (yc < Hh) & (xc >= 0) & (xc < Ww)).astype(np.float32)
        yi = np.clip(yc, 0, Hh - 1).astype(np.int64)
        xi = np.clip(xc, 0, Ww - 1).astype(np.int64)
        idx = (yi * Ww + xi).reshape(Bn, -1)
        gg = np.take_along_axis(flat, idx[:, None, :], axis=2)
        return gg.reshape(Bn, Cin, K2, Hh, Ww) * valid[:, None]

    v = (g(y0, x0) * ((1 - wy) * (1 - wx))[:, None]
         + g(y0, x0 + 1) * ((1 - wy) * wx)[:, None]
         + g(y0 + 1, x0) * (wy * (1 - wx))[:, None]
         + g(y0 + 1, x0 + 1) * (wy * wx)[:, None])
    out = np.einsum("bckhw,ock->bohw", v, w.reshape(w.shape[0], Cin, K2))
    return out + b[None, :, None, None]


def _window_radii(d):
    """Exact per-layer offset bounds + live L1 window terms for THIS input."""
    off1 = _conv3x3_np(d["x"], d["offset1_w"], d["offset1_b"])
    r1 = int(np.floor(np.abs(off1).max())) + 1
    h1 = _deform_np(d["x"], off1, d["conv1_w"], d["conv1_b"])
    s1 = d["bn1_gamma"] / np.sqrt(d["bn1_var"] + EPS)
    h1 = np.maximum(
        (h1 - d["bn1_mean"][None, :, None, None]) * s1[None, :, None, None]
        + d["bn1_beta"][None, :, None, None], 0.0)
    off2 = _conv3x3_np(h1, d["offset2_w"], d["offset2_b"])
    r2 = int(np.floor(np.abs(off2).max())) + 1
    # a window term (da, db) is live iff tri(dy-da)*tri(dx-db) > 0 somewhere
    o = off1.reshape(B, K2, 2, H, W)
    dy, dx = o[:, :, 0], o[:, :, 1]
    live = []
    for da in range(-r1, r1 + 1):
        for db in range(-r1, r1 + 1):
            if bool(np.any((np.abs(dy - da) < 1) & (np.abs(dx - db) < 1))):
                live.append((da, db))
    return r1, r2, live


# --------------------------------------------------------------------------
# device program
# --------------------------------------------------------------------------

def _build_program(r1, r2, live1=None):
    import concourse.bass as bass
    import concourse.tile as tile
    from concourse import mybir
    from concourse.vector_clock import ScopedClock

    f32 = mybir.dt.float32
    f32r = mybir.dt.float32r
    bf16 = mybir.dt.bfloat16
    AF = mybir.ActivationFunctionType
    ALU = mybir.AluOpType
    AX = mybir.AxisListType

    n1 = 2 * r1 + 1           # layer-1 window size per axis (5 expected)
    if live1 is None:
        live1 = [(a, b) for a in range(-r1, r1 + 1) for b in range(-r1, r1 + 1)]
    n2 = 2 * r2 + 1           # layer-2 window size per axis (3 expected)
    assert PAD1 >= 1 + r1 and PAD2 >= 1 + r2

    class SplitDrainTC(tile.TileContext):
        # The walrus build here accepts only one sync-wait per instruction on
        # several ISA structs.  Post-process every scheduled instruction and
        # move excess waits onto NOPs injected just before it (same engine
        # sequencer => identical semantics).
        MAXW = 1

        def _split_waits_in_blocks(self):
            nc = self.nc
            for f in nc.m.functions:
                for blk in f.blocks:
                    insts = blk.instructions
                    i = 0
                    while i < len(insts):
                        ins = insts[i]
                        si = ins.sync_info
                        if si is not None and si.on_wait and len(si.on_wait) > self.MAXW:
                            extra = list(si.on_wait[self.MAXW:])
                            si.on_wait = list(si.on_wait[:self.MAXW])
                            for w in extra:
                                nop = mybir.InstNoOp(
                                    name=f"I-{nc.next_id()}", ins=[], outs=[])
                                nop.engine = ins.engine
                                nop.sync_info = mybir.SyncInfo(
                                    on_wait=[w], on_update=[])
                                insts.insert(i, nop)
                                i += 1
                        i += 1

        def _drain_and_barrier(self, tick_clock, wait_clock):
            probe = self.nc.sync.nop()
            wait_clock.add_sem_waits(
                probe.ins, ScopedClock({None: tick_clock.global_clock}))
            waits = list(probe.ins.sync_info.on_wait or []) if probe.ins.sync_info else []
            if probe.ins.sync_info:
                probe.ins.sync_info.on_wait = waits[:1]
            for w in waits[1:]:
                n = self.nc.sync.nop()
                n.ins.sync_info = mybir.SyncInfo(on_wait=[w], on_update=[])
            self.nc.sync.drain()
            self.nc.all_engine_barrier()
            popped = self.nc._tile_sem_poison_stack.pop()
            assert popped is self._sem_poison
            self.nc.clear_and_free_semaphores(list(self.sems.allocated().values()))
            self.nc.all_engine_barrier()
            self._split_waits_in_blocks()

    nc = bass.Bass()

    x4 = nc.dram_tensor("x4", [BPC, C1, H, W], f32, kind="ExternalInput")
    # lhsT weights replicated along partitions so each image/sub block's
    # stationary operand sits on the same array rows as its moving operand
    w_off1 = nc.dram_tensor("w_off1", [C3, 18], f32, kind="ExternalInput")
    w_conv1 = nc.dram_tensor("w_conv1", [C3, C2], bf16, kind="ExternalInput")
    w_off2 = nc.dram_tensor("w_off2", [C3, K2, 18], bf16, kind="ExternalInput")
    w_conv2 = nc.dram_tensor("w_conv2", [C3, K2, C3], bf16, kind="ExternalInput")
    bn1v = nc.dram_tensor("bn1v", [C3, 2], f32, kind="ExternalInput")
    bn2v = nc.dram_tensor("bn2v", [C3, 2], f32, kind="ExternalInput")
    tyb1 = nc.dram_tensor("tyb1", [C3, 2 * n1], f32, kind="ExternalInput")
    tyb2 = nc.dram_tensor("tyb2", [18, 2 * n2], f32, kind="ExternalInput")
    fc1_lT = nc.dram_tensor("fc1_lT", [C3, 256], f32, kind="ExternalInput")
    fc1_b = nc.dram_tensor("fc1_b", [C3, 2], f32, kind="ExternalInput")
    fc2_lT = nc.dram_tensor("fc2_lT", [C3, 2, 200], f32, kind="ExternalInput")
    fc2_b = nc.dram_tensor("fc2_b", [100, 2], f32, kind="ExternalInput")
    y = nc.dram_tensor("y", [BPC, 200], f32, kind="ExternalOutput")

    def wap(sl, shift, rows, stride):
        """2D window AP over a padded free layout: [partition, rows, W]."""
        return bass.AP(tensor=sl.tensor, offset=sl.offset + shift,
                       ap=[sl.ap[0], [stride, rows], [1, W]])

    def rowbcast(t, row_off, nparts, nfree):
        # partition-broadcast read; src must be DRAM (SBUF needs nonzero step)
        a = t[:]
        return bass.AP(tensor=a.tensor, offset=a.offset + row_off,
                       ap=[[0, nparts], [1, nfree]])

    with SplitDrainTC(nc) as tc, ExitStack() as ctx:
        wp = ctx.enter_context(tc.tile_pool(name="wp", bufs=1))
        dpool = ctx.enter_context(tc.tile_pool(name="dram", bufs=1, space="DRAM"))

        t_woff1 = wp.tile([C3, 18], f32)
        nc.sync.dma_start(out=t_woff1[:], in_=w_off1[:])
        t_wconv1 = wp.tile([C3, C2], bf16)
        nc.sync.dma_start(out=t_wconv1[:], in_=w_conv1[:])
        t_woff2 = wp.tile([C3, K2, 18], bf16)
        nc.sync.dma_start(out=t_woff2[:], in_=w_off2[:])
        t_wconv2 = wp.tile([C3, K2, C3], bf16)
        nc.sync.dma_start(out=t_wconv2[:], in_=w_conv2[:])
        t_bn1 = wp.tile([C3, 2], f32)
        nc.sync.dma_start(out=t_bn1[:], in_=bn1v[:])
        t_bn2 = wp.tile([C3, 2], f32)
        nc.sync.dma_start(out=t_bn2[:], in_=bn2v[:])
        t_tyb1 = wp.tile([C3, 2 * n1], f32)
        nc.sync.dma_start(out=t_tyb1[:], in_=tyb1[:])
        t_tyb2 = wp.tile([18, 2 * n2], f32)
        nc.sync.dma_start(out=t_tyb2[:], in_=tyb2[:])
        t_fc1 = wp.tile([C3, 256], f32)
        nc.sync.dma_start(out=t_fc1[:], in_=fc1_lT[:])
        t_fc1b = wp.tile([C3, 2], f32)
        nc.sync.dma_start(out=t_fc1b[:], in_=fc1_b[:])
        t_fc2 = wp.tile([C3, 2, 200], f32)
        nc.sync.dma_start(out=t_fc2[:], in_=fc2_lT[:])
        t_fc2b = wp.tile([100, 2], f32)
        nc.sync.dma_start(out=t_fc2b[:], in_=fc2_b[:])

        h1p = wp.tile([C3, 2, HP2 * WP2], bf16)    # padded h1, pair-major
        nc.vector.memset(h1p[:], 0.0)
        pooled = wp.tile([C3, BPC], f32)

        # ============================ LAYER 1 ============================
        with tc.tile_pool(name="l1", bufs=1) as l1, \
             tc.tile_pool(name="l1b", bufs=2) as l1b, \
             tc.tile_pool(name="ps1", bufs=2, space="PSUM") as ps1:

            xrb = l1.tile([C3, HP1 * WP1], bf16)
            dyr = l1.tile([C3, HW], f32)
            dxr = l1.tile([C3, HW], f32)
            nc.vector.memset(dyr[:], 0.0)
            nc.vector.memset(dxr[:], 0.0)
            off1d = dpool.tile([BPC, 18, HW], f32, tag="off1d")

            with tc.tile_pool(name="l1x", bufs=1) as l1x:
                xrep = l1x.tile([C3, HP1 * WP1], f32)
                nc.vector.memset(xrep[:], 0.0)
                for img in range(BPC):
                    for k, (ki, kj) in enumerate(_taps):
                        sl = xrep[img * 32 + 3 * k: img * 32 + 3 * k + C1, :]
                        dst = wap(sl, (PAD1 - ki) * WP1 + (PAD1 - kj), H, WP1)
                        nc.sync.dma_start(out=dst, in_=x4[img])
                nc.vector.tensor_copy(out=xrb[:], in_=xrep[:])

                for img in range(BPC):
                    off1i = l1x.tile([18, HW], f32, tag="off1i")
                    for chk in range(NCHUNK):
                        ps = ps1.tile([18, CH], f32, tag="ps_a")
                        sl = xrep[img * 32: img * 32 + 32, :]
                        rhs = wap(sl, (PAD1 + chk * 8) * WP1 + PAD1, 8, WP1)
                        nc.tensor.matmul(ps[:],
                                         t_woff1[img * 32: img * 32 + 32, :],
                                         rhs, start=True, stop=True,
                                         tile_position=(img * 32, 0))
                        nc.scalar.copy(out=off1i[:, chk * CH:(chk + 1) * CH], in_=ps[:])
                    nc.sync.dma_start(out=off1d[img], in_=off1i[:])
                    for k in range(K2):
                        p0 = img * 32 + 3 * k
                        nc.sync.dma_start(
                            out=dyr[p0:p0 + C1, :],
                            in_=rowbcast(off1d, (img * 18 + 2 * k) * HW, C1, HW))
                        nc.sync.dma_start(
                            out=dxr[p0:p0 + C1, :],
                            in_=rowbcast(off1d, (img * 18 + 2 * k + 1) * HW, C1, HW))

            U1 = l1.tile([C3, HW], f32)
            ty = l1.tile([C3, HW], bf16)
            first = True
            nterm = 0
            for ia in range(n1):
                tyf = l1b.tile([C3, HW], f32, tag="tyf")
                nc.scalar.activation(out=tyf[:], in_=dyr[:], func=AF.Abs,
                                     bias=t_tyb1[:, ia:ia + 1], scale=1.0)
                nc.scalar.activation(out=ty[:], in_=tyf[:], func=AF.Relu,
                                     bias=1.0, scale=-1.0)
                for ib in range(n1):
                    if (ia - r1, ib - r1) not in live1:
                        continue
                    txf = l1b.tile([C3, HW], f32, tag="tyf")
                    tx = l1b.tile([C3, HW], bf16, tag="tx1")
                    nc.scalar.activation(out=txf[:], in_=dxr[:], func=AF.Abs,
                                         bias=t_tyb1[:, n1 + ib:n1 + ib + 1], scale=1.0)
                    nc.scalar.activation(out=tx[:], in_=txf[:], func=AF.Relu,
                                         bias=1.0, scale=-1.0)
                    psi = l1b.tile([C3, HW], bf16, tag="psi1")
                    if "l1mod" in _SKIP:
                        continue
                    nc.vector.tensor_mul(out=psi[:], in0=ty[:], in1=tx[:])
                    xw = wap(xrb[:], (PAD1 + ia - r1) * WP1 + (PAD1 + ib - r1), H, WP1)
                    nc.vector.tensor_mul(out=psi[:], in0=psi[:], in1=xw)
                    if first:
                        nc.vector.tensor_copy(out=U1[:], in_=psi[:])
                        first = False
                    else:
                        nc.vector.tensor_add(out=U1[:], in0=U1[:], in1=psi[:])
                    nterm += 1

            U1b = l1.tile([C3, HW], bf16)
            nc.vector.tensor_copy(out=U1b[:], in_=U1[:])
            for img in range(BPC):
                pair, sub = img // 2, img % 2
                for chk in range(NCHUNK):
                    ps = ps1.tile([C2, CH], f32, tag="ps_b")
                    nc.tensor.matmul(
                        ps[:],
                        t_wconv1[img * 32: img * 32 + 32, :],
                        U1b[img * 32: img * 32 + 32,
                            chk * CH:(chk + 1) * CH],
                        start=True, stop=True, tile_position=(img * 32, 0))
                    dsl = h1p[sub * C2:(sub + 1) * C2, pair, :]
                    dst = wap(dsl, (PAD2 + chk * 8) * WP2 + PAD2, 8, WP2)
                    nc.scalar.activation(
                        out=dst, in_=ps[:], func=AF.Relu,
                        bias=t_bn1[sub * C2:(sub + 1) * C2, 1:2],
                        scale=t_bn1[sub * C2:(sub + 1) * C2, 0:1])

        # ============================ LAYER 2 ============================
        for pair in range(2):
            with tc.tile_pool(name=f"l2t_{pair}", bufs=1) as l2t, \
                 tc.tile_pool(name=f"ps2_{pair}", bufs=2, space="PSUM") as ps2:
                ty2 = l2t.tile([18, n2, HW], bf16)
                tx2 = l2t.tile([18, n2, HW], bf16)

                with tc.tile_pool(name=f"l2a_{pair}", bufs=1) as l2a:
                    dy2 = l2a.tile([18, HW], f32)
                    dx2 = l2a.tile([18, HW], f32)
                    for sub in range(2):
                        off2i = l2a.tile([18, HW], f32, tag="off2i")
                        for chk in range(NCHUNK):
                            ps = ps2.tile([18, CH], f32, tag="ps_a")
                            for k, (ki, kj) in enumerate(_taps):
                                sl = h1p[sub * C2:(sub + 1) * C2, pair, :]
                                rhs = wap(sl, (PAD2 + chk * 8 + ki) * WP2 + PAD2 + kj,
                                          8, WP2)
                                nc.tensor.matmul(ps[:],
                                                 t_woff2[sub * C2:(sub + 1) * C2, k, :],
                                                 rhs,
                                                 start=(k == 0), stop=(k == K2 - 1),
                                                 tile_position=(sub * C2, 0))
                            nc.scalar.copy(out=off2i[:, chk * CH:(chk + 1) * CH],
                                           in_=ps[:])
                        for k in range(K2):
                            nc.sync.dma_start(
                                out=dy2[sub * K2 + k: sub * K2 + k + 1, :],
                                in_=off2i[2 * k: 2 * k + 1, :])
                            nc.sync.dma_start(
                                out=dx2[sub * K2 + k: sub * K2 + k + 1, :],
                                in_=off2i[2 * k + 1: 2 * k + 2, :])

                    for ia in range(n2):
                        tmp = l2a.tile([18, HW], f32, tag="tmp2")
                        nc.scalar.activation(out=tmp[:], in_=dy2[:], func=AF.Abs,
                                             bias=t_tyb2[:, ia:ia + 1], scale=1.0)
                        nc.scalar.activation(out=ty2[:, ia, :], in_=tmp[:], func=AF.Relu,
                                             bias=1.0, scale=-1.0)
                        tmp2 = l2a.tile([18, HW], f32, tag="tmp2")
                        nc.scalar.activation(out=tmp2[:], in_=dx2[:], func=AF.Abs,
                                             bias=t_tyb2[:, n2 + ia:n2 + ia + 1],
                                             scale=1.0)
                        nc.scalar.activation(out=tx2[:, ia, :], in_=tmp2[:], func=AF.Relu,
                                             bias=1.0, scale=-1.0)

                with tc.tile_pool(name=f"l2m_{pair}", bufs=1) as l2m, \
                     tc.tile_pool(name=f"l2tr_{pair}", bufs=2) as l2tr:
                    # all 9 tri-products to DRAM once; broadcast-read per tap
                    psid = dpool.tile([n2 * n2, 18, HW], bf16, tag="psid")
                    for ia in range(n2):
                        for ib in range(n2):
                            psi2 = l2m.tile([18, HW], bf16, tag="psi2")
                            nc.vector.tensor_mul(out=psi2[:], in0=ty2[:, ia, :],
                                                 in1=tx2[:, ib, :])
                            nc.sync.dma_start(out=psid[ia * n2 + ib], in_=psi2[:])

                    h2acc = l2m.tile([C3, 2, HW], f32)
                    for k, (ki, kj) in enumerate(_taps):
                        # two bf16 half-accumulators shorten the serial chain
                        A1 = l2tr.tile([C3, HW], bf16, tag="A1")
                        A2 = l2tr.tile([C3, HW], bf16, tag="A2")
                        na1 = na2 = 0
                        nterm = 0
                        for ia in range(n2):
                            for ib in range(n2):
                                if "l2mod" in _SKIP:
                                    continue
                                psir = l2tr.tile([C3, HW], bf16, tag="psir")
                                for sub in range(2):
                                    nc.sync.dma_start(
                                        out=psir[sub * C2:(sub + 1) * C2, :],
                                        in_=rowbcast(
                                            psid,
                                            ((ia * n2 + ib) * 18 + sub * K2 + k) * HW,
                                            C2, HW))
                                hwa = bass.AP(
                                    tensor=h1p[:].tensor,
                                    offset=h1p[:].offset + pair * (HP2 * WP2)
                                    + (PAD2 + ki + ia - r2) * WP2
                                    + (PAD2 + kj + ib - r2),
                                    ap=[h1p[:].ap[0], [WP2, H], [1, W]])
                                if "l2dve" in _SKIP:
                                    continue
                                P = l2tr.tile([C3, HW], bf16, tag="P2")
                                nc.vector.tensor_mul(out=P[:], in0=psir[:], in1=hwa)
                                if nterm % 2 == 0:
                                    if na1 == 0:
                                        nc.vector.tensor_copy(out=A1[:], in_=P[:])
                                    else:
                                        nc.vector.tensor_add(out=A1[:], in0=A1[:], in1=P[:])
                                    na1 += 1
                                else:
                                    if na2 == 0:
                                        nc.vector.tensor_copy(out=A2[:], in_=P[:])
                                    else:
                                        nc.vector.tensor_add(out=A2[:], in0=A2[:], in1=P[:])
                                    na2 += 1
                                nterm += 1
                        nc.vector.tensor_add(out=A1[:], in0=A1[:], in1=A2[:])
                        for sub in range(2):
                            for chk in range(NCHUNK):
                                ps = ps2.tile([C3, CH], f32, tag="ps_b")
                                nc.tensor.matmul(
                                    ps[:],
                                    t_wconv2[sub * C2:(sub + 1) * C2, k, :],
                                    A1[sub * C2:(sub + 1) * C2,
                                       chk * CH:(chk + 1) * CH],
                                    start=True, stop=True, tile_position=(sub * C2, 0))
                                dst = h2acc[:, sub, chk * CH:(chk + 1) * CH]
                                if k == 0:
                                    nc.vector.tensor_copy(out=dst, in_=ps[:])
                                else:
                                    nc.vector.tensor_add(out=dst, in0=dst, in1=ps[:])

                    for sub in range(2):
                        img = pair * 2 + sub
                        h2 = l2m.tile([C3, HW], bf16, tag="h2")
                        nc.scalar.activation(out=h2[:], in_=h2acc[:, sub, :],
                                             func=AF.Relu,
                                             bias=t_bn2[:, 1:2], scale=t_bn2[:, 0:1])
                        nc.vector.tensor_reduce(out=pooled[:, img:img + 1], in_=h2[:],
                                                axis=AX.X, op=ALU.add)

        # ============================ FC head ============================
        with tc.tile_pool(name="fc", bufs=1) as fcp, \
             tc.tile_pool(name="psf", bufs=2, space="PSUM") as psf:
            y1 = fcp.tile([C3, 2, BPC], f32)
            for mh in range(2):
                ps = psf.tile([C3, BPC], f32, tag="f1")
                nc.tensor.matmul(ps[:],
                                 t_fc1[:, mh * C3:(mh + 1) * C3],
                                 pooled[:], start=True, stop=True)
                nc.scalar.activation(out=y1[:, mh, :], in_=ps[:], func=AF.Relu,
                                     bias=t_fc1b[:, mh:mh + 1], scale=1.0)
            yout = fcp.tile([100, 2, BPC], f32)
            for mh in range(2):
                ps = psf.tile([100, BPC], f32, tag="f2")
                for kc in range(2):
                    nc.tensor.matmul(ps[:],
                                     t_fc2[:, kc, mh * 100:(mh + 1) * 100],
                                     y1[:, kc, :],
                                     start=(kc == 0), stop=(kc == 1))
                nc.vector.tensor_scalar(out=yout[:, mh, :], in0=ps[:],
                                        scalar1=t_fc2b[:, mh:mh + 1], scalar2=None,
                                        op0=ALU.add)
            for mh in range(2):
                ya = y[:]
                dst = bass.AP(tensor=ya.tensor, offset=ya.offset + mh * 100,
                              ap=[[1, 100], [200, BPC]])
                nc.sync.dma_start(out=dst, in_=yout[:, mh, :])

    return nc


# --------------------------------------------------------------------------
# host prep + entry point
# --------------------------------------------------------------------------

def _host_prep(d, r1, r2):
    import ml_dtypes
    n1, n2 = 2 * r1 + 1, 2 * r2 + 1

    w_off1 = np.zeros((32, 18), np.float32)
    w_conv1 = np.zeros((32, C2), np.float32)
    for k in range(K2):
        ki, kj = k // 3, k % 3
        for c in range(C1):
            w_off1[k * 3 + c, :] = d["offset1_w"][:, c, ki, kj]
            w_conv1[k * 3 + c, :] = d["conv1_w"][:, c, ki, kj]
    w_off1 = np.tile(w_off1, (4, 1))            # replicate per image block
    w_conv1 = np.tile(w_conv1, (4, 1))

    w_off2 = np.zeros((C2, K2, 18), np.float32)
    w_conv2 = np.zeros((C2, K2, C3), np.float32)
    for k in range(K2):
        ki, kj = k // 3, k % 3
        w_off2[:, k, :] = d["offset2_w"][:, :, ki, kj].T
        w_conv2[:, k, :] = d["conv2_w"][:, :, ki, kj].T
    w_off2 = np.tile(w_off2, (2, 1, 1))         # replicate per image sub block
    w_conv2 = np.tile(w_conv2, (2, 1, 1))

    s1 = d["bn1_gamma"] / np.sqrt(d["bn1_var"] + EPS)
    t1 = (d["conv1_b"] - d["bn1_mean"]) * s1 + d["bn1_beta"]
    bn1v = np.stack([np.tile(s1, 2), np.tile(t1, 2)], axis=1)     # [128, 2]
    s2 = d["bn2_gamma"] / np.sqrt(d["bn2_var"] + EPS)
    t2 = (d["conv2_b"] - d["bn2_mean"]) * s2 + d["bn2_beta"]
    bn2v = np.stack([s2, t2], axis=1)                             # [128, 2]

    b1 = d["offset1_b"]
    tyb1 = np.zeros((C3, 2 * n1), np.float32)
    for r in range(C3):
        k = min((r % 32) // 3, K2 - 1)
        for i in range(n1):
            tyb1[r, i] = b1[2 * k] - (i - r1)
            tyb1[r, n1 + i] = b1[2 * k + 1] - (i - r1)
    b2 = d["offset2_b"]
    tyb2 = np.zeros((18, 2 * n2), np.float32)
    for r in range(18):
        k = r % K2
        for i in range(n2):
            tyb2[r, i] = b2[2 * k] - (i - r2)
            tyb2[r, n2 + i] = b2[2 * k + 1] - (i - r2)

    return {
        "w_off1": w_off1, "w_conv1": w_conv1.astype(ml_dtypes.bfloat16),
        "w_off2": w_off2.astype(ml_dtypes.bfloat16),
        "w_conv2": w_conv2.astype(ml_dtypes.bfloat16),
        "bn1v": bn1v, "bn2v": bn2v, "tyb1": tyb1, "tyb2": tyb2,
        "fc1_lT": np.ascontiguousarray((d["fc1_w"] / HW).T),
        "fc1_b": np.ascontiguousarray(d["fc1_b"].reshape(2, C3).T),
        "fc2_lT": np.ascontiguousarray(
            d["fc2_w"].T.reshape(2, C3, 200).transpose(1, 0, 2)),
        "fc2_b": np.ascontiguousarray(d["fc2_b"].reshape(2, 100).T),
    }


def kernel(**inputs):
    global _LAST_RESULTS
    from concourse.bass_utils import run_bass_kernel_spmd

    d = {k: np.asarray(v, np.float32) for k, v in inputs.items()}
    r1, r2, live1 = _window_radii(d)
    assert r1 <= 2 and r2 <= 2, (r1, r2)

    base = _host_prep(d, r1, r2)
    nc = _build_program(r1, r2, live1)

    x = d["x"]
    in_maps = []
    for core in range(NCORES):
        m = dict(base)
        m["x4"] = np.ascontiguousarray(x[core * BPC:(core + 1) * BPC])
        in_maps.append(m)
    res = run_bass_kernel_spmd(nc, in_maps, core_ids=list(range(NCORES)),
                               trace=_TRACE)
    _LAST_RESULTS = res
    return np.concatenate([r["y"] for r in res.results], axis=0).astype(np.float32)


# revision 34
# speedup vs baseline: 1.0048x; 1.0048x over previous
"""Trainium2 Bass kernel for DeformableTinyImageNet (32x3x64x64 -> 32x200).

Sharding: data-parallel, 32 images -> 8 cores x 4 images; all weights
replicated (tiny vs activations).

Deformable conv is computed exactly as a *modulated convolution*: for
offsets |d| < R, the bilinear sample at (h+ki+dy, w+kj+dx) equals
    sum_{da,db, |da|,|db|<=R} tri(dy-da)*tri(dx-db) * x[c, h+ki+da, w+kj+db]
with tri(t) = max(0, 1-|t|)  (tri has unit support, so the window covers
all corners).  Window radii are validated on the host against the exact
offset fields (offsets are deterministic functions of the inputs).

Layer 1 (Cin=3): partitions pack (img, tap, channel) = 4*(9*3 -> 32) with
the tap shift pre-baked into each replicated row of a padded input copy,
so tri fields are built directly in replicated row space.
Layer 2 (Cin=64): partitions pack (2 images x 64 channels); per-tap tri
products are built in position space [18, HW] and broadcast to the 128
modulate rows with partition-stride-0 DMAs.
"""

import numpy as np
from contextlib import ExitStack

B, C1, H, W = 32, 3, 64, 64
HW = H * W
NCORES = 8
BPC = B // NCORES
C2 = 64
C3 = 128
K2 = 9
EPS = 1e-5

PAD1 = 3
WP1, HP1 = W + 2 * PAD1, H + 2 * PAD1        # 70
PAD2 = 2
WP2, HP2 = W + 2 * PAD2, H + 2 * PAD2        # 68

NCHUNK = 8
CH = HW // NCHUNK                             # 512

_taps = [(ki, kj) for ki in (-1, 0, 1) for kj in (-1, 0, 1)]

# test.py hooks
_SKIP = set()
_TRACE = False
_LAST_RESULTS = None


# --------------------------------------------------------------------------
# host-side numpy helpers (window validation only; all NN math runs on device)
# --------------------------------------------------------------------------

def _conv3x3_np(x, w, b):
    Bn, C, Hh, Ww = x.shape
    xp = np.pad(x, ((0, 0), (0, 0), (1, 1), (1, 1)))
    out = np.zeros((Bn, w.shape[0], Hh, Ww), np.float32)
    for ki in range(3):
        for kj in range(3):
            out += np.einsum("oc,bchw->bohw", w[:, :, ki, kj],
                             xp[:, :, ki:ki + Hh, kj:kj + Ww])
    return out + b[None, :, None, None]


def _deform_np(x, off, w, b):
    Bn, Cin, Hh, Ww = x.shape
    o = off.reshape(Bn, K2, 2, Hh, Ww)
    dy, dx = o[:, :, 0], o[:, :, 1]
    ki = (np.arange(3).repeat(3) - 1).astype(np.float32)
    kj = (np.tile(np.arange(3), 3) - 1).astype(np.float32)
    py = np.arange(Hh, dtype=np.float32)[None, None, :, None] + ki[None, :, None, None] + dy
    px = np.arange(Ww, dtype=np.float32)[None, None, None, :] + kj[None, :, None, None] + dx
    y0, x0 = np.floor(py), np.floor(px)
    wy, wx = py - y0, px - x0
    flat = x.reshape(Bn, Cin, Hh * Ww)

    def g(yc, xc):
        valid = ((yc >= 0) & (yc < Hh) & (xc >= 0) & (xc < Ww)).astype(np.float32)
        yi = np.clip(yc, 0, Hh - 1).astype(np.int64)
        xi = np.clip(xc, 0, Ww - 1).astype(np.int64)
        idx = (yi * Ww + xi).reshape(Bn, -1)
        gg = np.take_along_axis(flat, idx[:, None, :], axis=2)
        return gg.reshape(Bn, Cin, K2, Hh, Ww) * valid[:, None]

    v = (g(y0, x0) * ((1 - wy) * (1 - wx))[:, None]
         + g(y0, x0 + 1) * ((1 - wy) * wx)[:, None]
         + g(y0 + 1, x0) * (wy * (1 - wx))[:, None]
         + g(y0 + 1, x0 + 1) * (wy * wx)[:, None])
    out = np.einsum("bckhw,ock->bohw", v, w.reshape(w.shape[0], Cin, K2))
    return out + b[None, :, None, None]


def _window_radii(d):
    """Exact per-layer offset bounds + live L1 window terms for THIS input."""
    off1 = _conv3x3_np(d["x"], d["offset1_w"], d["offset1_b"])
    r1 = int(np.floor(np.abs(off1).max())) + 1
    h1 = _deform_np(d["x"], off1, d["conv1_w"], d["conv1_b"])
    s1 = d["bn1_gamma"] / np.sqrt(d["bn1_var"] + EPS)
    h1 = np.maximum(
        (h1 - d["bn1_mean"][None, :, None, None]) * s1[None, :, None, None]
        + d["bn1_beta"][None, :, None, None], 0.0)
    off2 = _conv3x3_np(h1, d["offset2_w"], d["offset2_b"])
    r2 = int(np.floor(np.abs(off2).max())) + 1
    # a window term (da, db) is live iff tri(dy-da)*tri(dx-db) > 0 somewhere
    o = off1.reshape(B, K2, 2, H, W)
    dy, dx = o[:, :, 0], o[:, :, 1]
    live = []
    for da in range(-r1, r1 + 1):
        for db in range(-r1, r1 + 1):
            if bool(np.any((np.abs(dy - da) < 1) & (np.abs(dx - db) < 1))):
                live.append((da, db))
    return r1, r2, live


# --------------------------------------------------------------------------
# device program
# --------------------------------------------------------------------------

def _build_program(r1, r2, live1=None):
    import concourse.bass as bass
    import concourse.tile as tile
    from concourse import mybir
    from concourse.vector_clock import ScopedClock

    f32 = mybir.dt.float32
    f32r = mybir.dt.float32r
    bf16 = mybir.dt.bfloat16
    AF = mybir.ActivationFunctionType
    ALU = mybir.AluOpType
    AX = mybir.AxisListType

    n1 = 2 * r1 + 1           # layer-1 window size per axis (5 expected)
    if live1 is None:
        live1 = [(a, b) for a in range(-r1, r1 + 1) for b in range(-r1, r1 + 1)]
    n2 = 2 * r2 + 1           # layer-2 window size per axis (3 expected)
    assert PAD1 >= 1 + r1 and PAD2 >= 1 + r2

    class SplitDrainTC(tile.TileContext):
        # The walrus build here accepts only one sync-wait per instruction on
        # several ISA structs.  Post-process every scheduled instruction and
        # move excess waits onto NOPs injected just before it (same engine
        # sequencer => identical semantics).
        MAXW = 1

        def _split_waits_in_blocks(self):
            nc = self.nc
            for f in nc.m.functions:
                for blk in f.blocks:
                    insts = blk.instructions
                    i = 0
                    while i < len(insts):
                        ins = insts[i]
                        si = ins.sync_info
                        if si is not None and si.on_wait and len(si.on_wait) > self.MAXW:
                            extra = list(si.on_wait[self.MAXW:])
                            si.on_wait = list(si.on_wait[:self.MAXW])
                            for w in extra:
                                nop = mybir.InstNoOp(
                                    name=f"I-{nc.next_id()}", ins=[], outs=[])
                                nop.engine = ins.engine
                                nop.sync_info = mybir.SyncInfo(
                                    on_wait=[w], on_update=[])
                                insts.insert(i, nop)
                                i += 1
                        i += 1

        def _drain_and_barrier(self, tick_clock, wait_clock):
            probe = self.nc.sync.nop()
            wait_clock.add_sem_waits(
                probe.ins, ScopedClock({None: tick_clock.global_clock}))
            waits = list(probe.ins.sync_info.on_wait or []) if probe.ins.sync_info else []
            if probe.ins.sync_info:
                probe.ins.sync_info.on_wait = waits[:1]
            for w in waits[1:]:
                n = self.nc.sync.nop()
                n.ins.sync_info = mybir.SyncInfo(on_wait=[w], on_update=[])
            self.nc.sync.drain()
            self.nc.all_engine_barrier()
            popped = self.nc._tile_sem_poison_stack.pop()
            assert popped is self._sem_poison
            self.nc.clear_and_free_semaphores(list(self.sems.allocated().values()))
            self.nc.all_engine_barrier()
            self._split_waits_in_blocks()

    nc = bass.Bass()

    x4 = nc.dram_tensor("x4", [BPC, C1, H, W], f32, kind="ExternalInput")
    # lhsT weights replicated along partitions so each image/sub block's
    # stationary operand sits on the same array rows as its moving operand
    w_off1 = nc.dram_tensor("w_off1", [C3, 18], f32, kind="ExternalInput")
    w_conv1 = nc.dram_tensor("w_conv1", [C3, C2], bf16, kind="ExternalInput")
    w_off2 = nc.dram_tensor("w_off2", [C3, K2, 18], bf16, kind="ExternalInput")
    w_conv2 = nc.dram_tensor("w_conv2", [C3, K2, C3], bf16, kind="ExternalInput")
    bn1v = nc.dram_tensor("bn1v", [C3, 2], f32, kind="ExternalInput")
    bn2v = nc.dram_tensor("bn2v", [C3, 2], f32, kind="ExternalInput")
    tyb1 = nc.dram_tensor("tyb1", [C3, 2 * n1], f32, kind="ExternalInput")
    tyb2 = nc.dram_tensor("tyb2", [18, 2 * n2], f32, kind="ExternalInput")
    fc1_lT = nc.dram_tensor("fc1_lT", [C3, 256], f32, kind="ExternalInput")
    fc1_b = nc.dram_tensor("fc1_b", [C3, 2], f32, kind="ExternalInput")
    fc2_lT = nc.dram_tensor("fc2_lT", [C3, 2, 200], f32, kind="ExternalInput")
    fc2_b = nc.dram_tensor("fc2_b", [100, 2], f32, kind="ExternalInput")
    y = nc.dram_tensor("y", [BPC, 200], f32, kind="ExternalOutput")

    def wap(sl, shift, rows, stride):
        """2D window AP over a padded free layout: [partition, rows, W]."""
        return bass.AP(tensor=sl.tensor, offset=sl.offset + shift,
                       ap=[sl.ap[0], [stride, rows], [1, W]])

    def rowbcast(t, row_off, nparts, nfree):
        # partition-broadcast read; src must be DRAM (SBUF needs nonzero step)
        a = t[:]
        return bass.AP(tensor=a.tensor, offset=a.offset + row_off,
                       ap=[[0, nparts], [1, nfree]])

    with SplitDrainTC(nc) as tc, ExitStack() as ctx:
        wp = ctx.enter_context(tc.tile_pool(name="wp", bufs=1))
        dpool = ctx.enter_context(tc.tile_pool(name="dram", bufs=1, space="DRAM"))

        t_woff1 = wp.tile([C3, 18], f32)
        nc.sync.dma_start(out=t_woff1[:], in_=w_off1[:])
        t_wconv1 = wp.tile([C3, C2], bf16)
        nc.sync.dma_start(out=t_wconv1[:], in_=w_conv1[:])
        t_woff2 = wp.tile([C3, K2, 18], bf16)
        nc.sync.dma_start(out=t_woff2[:], in_=w_off2[:])
        t_wconv2 = wp.tile([C3, K2, C3], bf16)
        nc.sync.dma_start(out=t_wconv2[:], in_=w_conv2[:])
        t_bn1 = wp.tile([C3, 2], f32)
        nc.sync.dma_start(out=t_bn1[:], in_=bn1v[:])
        t_bn2 = wp.tile([C3, 2], f32)
        nc.sync.dma_start(out=t_bn2[:], in_=bn2v[:])
        t_tyb1 = wp.tile([C3, 2 * n1], f32)
        nc.sync.dma_start(out=t_tyb1[:], in_=tyb1[:])
        t_tyb2 = wp.tile([18, 2 * n2], f32)
        nc.sync.dma_start(out=t_tyb2[:], in_=tyb2[:])
        t_fc1 = wp.tile([C3, 256], f32)
        nc.sync.dma_start(out=t_fc1[:], in_=fc1_lT[:])
        t_fc1b = wp.tile([C3, 2], f32)
        nc.sync.dma_start(out=t_fc1b[:], in_=fc1_b[:])
        t_fc2 = wp.tile([C3, 2, 200], f32)
        nc.sync.dma_start(out=t_fc2[:], in_=fc2_lT[:])
        t_fc2b = wp.tile([100, 2], f32)
        nc.sync.dma_start(out=t_fc2b[:], in_=fc2_b[:])

        h1p = wp.tile([C3, 2, HP2 * WP2], bf16)    # padded h1, pair-major
        nc.vector.memset(h1p[:], 0.0)
        pooled = wp.tile([C3, BPC], f32)

        # ============================ LAYER 1 ============================
        with tc.tile_pool(name="l1", bufs=1) as l1, \
             tc.tile_pool(name="l1b", bufs=2) as l1b, \
             tc.tile_pool(name="ps1", bufs=2, space="PSUM") as ps1:

            xrb = l1.tile([C3, HP1 * WP1], bf16)
            dyr = l1.tile([C3, HW], f32)
            dxr = l1.tile([C3, HW], f32)
            nc.vector.memset(dyr[:], 0.0)
            nc.vector.memset(dxr[:], 0.0)
            off1d = dpool.tile([BPC, 18, HW], f32, tag="off1d")

            with tc.tile_pool(name="l1x", bufs=1) as l1x:
                xrep = l1x.tile([C3, HP1 * WP1], f32)
                nc.vector.memset(xrep[:], 0.0)
                for img in range(BPC):
                    for k, (ki, kj) in enumerate(_taps):
                        sl = xrep[img * 32 + 3 * k: img * 32 + 3 * k + C1, :]
                        dst = wap(sl, (PAD1 - ki) * WP1 + (PAD1 - kj), H, WP1)
                        nc.sync.dma_start(out=dst, in_=x4[img])
                nc.vector.tensor_copy(out=xrb[:], in_=xrep[:])

                for img in range(BPC):
                    off1i = l1x.tile([18, HW], f32, tag="off1i")
                    for chk in range(NCHUNK):
                        ps = ps1.tile([18, CH], f32, tag="ps_a")
                        sl = xrep[img * 32: img * 32 + 32, :]
                        rhs = wap(sl, (PAD1 + chk * 8) * WP1 + PAD1, 8, WP1)
                        nc.tensor.matmul(ps[:],
                                         t_woff1[img * 32: img * 32 + 32, :],
                                         rhs, start=True, stop=True,
                                         tile_position=(img * 32, 0))
                        nc.scalar.copy(out=off1i[:, chk * CH:(chk + 1) * CH], in_=ps[:])
                    nc.sync.dma_start(out=off1d[img], in_=off1i[:])
                    for k in range(K2):
                        p0 = img * 32 + 3 * k
                        nc.sync.dma_start(
                            out=dyr[p0:p0 + C1, :],
                            in_=rowbcast(off1d, (img * 18 + 2 * k) * HW, C1, HW))
                        nc.sync.dma_start(
                            out=dxr[p0:p0 + C1, :],
                            in_=rowbcast(off1d, (img * 18 + 2 * k + 1) * HW, C1, HW))

            U1 = l1.tile([C3, HW], f32)
            ty = l1.tile([C3, HW], bf16)
            nu = [0]
            nterm = 0
            for ia in range(n1):
                tyf = l1b.tile([C3, HW], f32, tag="tyf")
                nc.scalar.activation(out=tyf[:], in_=dyr[:], func=AF.Abs,
                                     bias=t_tyb1[:, ia:ia + 1], scale=1.0)
                nc.scalar.activation(out=ty[:], in_=tyf[:], func=AF.Relu,
                                     bias=1.0, scale=-1.0)
                for ib in range(n1):
                    if (ia - r1, ib - r1) not in live1:
                        continue
                    txf = l1b.tile([C3, HW], f32, tag="tyf")
                    tx = l1b.tile([C3, HW], bf16, tag="tx1")
                    nc.scalar.activation(out=txf[:], in_=dxr[:], func=AF.Abs,
                                         bias=t_tyb1[:, n1 + ib:n1 + ib + 1], scale=1.0)
                    nc.scalar.activation(out=tx[:], in_=txf[:], func=AF.Relu,
                                         bias=1.0, scale=-1.0)
                    psi = l1b.tile([C3, HW], bf16, tag="psi1")
                    if "l1mod" in _SKIP:
                        continue
                    nc.vector.tensor_mul(out=psi[:], in0=ty[:], in1=tx[:])
                    xw = wap(xrb[:], (PAD1 + ia - r1) * WP1 + (PAD1 + ib - r1), H, WP1)
                    nc.vector.tensor_mul(out=psi[:], in0=psi[:], in1=xw)
                    if nu[0] == 0:
                        nc.vector.tensor_copy(out=U1[:], in_=psi[:])
                    else:
                        nc.vector.tensor_add(out=U1[:], in0=U1[:], in1=psi[:])
                    nu[0] += 1
                    nterm += 1

            U1b = l1.tile([C3, HW], bf16)
            nc.vector.tensor_copy(out=U1b[:], in_=U1[:])
            for img in range(BPC):
                pair, sub = img // 2, img % 2
                for chk in range(NCHUNK):
                    ps = ps1.tile([C2, CH], f32, tag="ps_b")
                    nc.tensor.matmul(
                        ps[:],
                        t_wconv1[img * 32: img * 32 + 32, :],
                        U1b[img * 32: img * 32 + 32,
                            chk * CH:(chk + 1) * CH],
                        start=True, stop=True, tile_position=(img * 32, 0))
                    dsl = h1p[sub * C2:(sub + 1) * C2, pair, :]
                    dst = wap(dsl, (PAD2 + chk * 8) * WP2 + PAD2, 8, WP2)
                    nc.scalar.activation(
                        out=dst, in_=ps[:], func=AF.Relu,
                        bias=t_bn1[sub * C2:(sub + 1) * C2, 1:2],
                        scale=t_bn1[sub * C2:(sub + 1) * C2, 0:1])

        # ============================ LAYER 2 ============================
        for pair in range(2):
            with tc.tile_pool(name=f"l2t_{pair}", bufs=1) as l2t, \
                 tc.tile_pool(name=f"ps2_{pair}", bufs=2, space="PSUM") as ps2:
                ty2 = l2t.tile([18, n2, HW], bf16)
                tx2 = l2t.tile([18, n2, HW], bf16)

                with tc.tile_pool(name=f"l2a_{pair}", bufs=1) as l2a:
                    dy2 = l2a.tile([18, HW], f32)
                    dx2 = l2a.tile([18, HW], f32)
                    for sub in range(2):
                        off2i = l2a.tile([18, HW], f32, tag="off2i")
                        for chk in range(NCHUNK):
                            ps = ps2.tile([18, CH], f32, tag="ps_a")
                            for k, (ki, kj) in enumerate(_taps):
                                sl = h1p[sub * C2:(sub + 1) * C2, pair, :]
                                rhs = wap(sl, (PAD2 + chk * 8 + ki) * WP2 + PAD2 + kj,
                                          8, WP2)
                                nc.tensor.matmul(ps[:],
                                                 t_woff2[sub * C2:(sub + 1) * C2, k, :],
                                                 rhs,
                                                 start=(k == 0), stop=(k == K2 - 1),
                                                 tile_position=(sub * C2, 0))
                            nc.scalar.copy(out=off2i[:, chk * CH:(chk + 1) * CH],
                                           in_=ps[:])
                        for k in range(K2):
                            nc.sync.dma_start(
                                out=dy2[sub * K2 + k: sub * K2 + k + 1, :],
                                in_=off2i[2 * k: 2 * k + 1, :])
                            nc.sync.dma_start(
                                out=dx2[sub * K2 + k: sub * K2 + k + 1, :],
                                in_=off2i[2 * k + 1: 2 * k + 2, :])

                    for ia in range(n2):
                        tmp = l2a.tile([18, HW], f32, tag="tmp2")
                        nc.scalar.activation(out=tmp[:], in_=dy2[:], func=AF.Abs,
                                             bias=t_tyb2[:, ia:ia + 1], scale=1.0)
                        nc.scalar.activation(out=ty2[:, ia, :], in_=tmp[:], func=AF.Relu,
                                             bias=1.0, scale=-1.0)
                        tmp2 = l2a.tile([18, HW], f32, tag="tmp2")
                        nc.scalar.activation(out=tmp2[:], in_=dx2[:], func=AF.Abs,
                                             bias=t_tyb2[:, n2 + ia:n2 + ia + 1],
                                             scale=1.0)
                        nc.scalar.activation(out=tx2[:, ia, :], in_=tmp2[:], func=AF.Relu,
                                             bias=1.0, scale=-1.0)

                with tc.tile_pool(name=f"l2m_{pair}", bufs=1) as l2m, \
                     tc.tile_pool(name=f"l2tr_{pair}", bufs=2) as l2tr, \
                     tc.tile_pool(name=f"ps2b_{pair}", bufs=4, space="PSUM") as ps2b:
                    # all 9 tri-products to DRAM once; broadcast-read per tap
                    psid = dpool.tile([n2 * n2, 18, HW], bf16, tag="psid")
                    for ia in range(n2):
                        for ib in range(n2):
                            psi2 = l2m.tile([18, HW], bf16, tag="psi2")
                            nc.vector.tensor_mul(out=psi2[:], in0=ty2[:, ia, :],
                                                 in1=tx2[:, ib, :])
                            nc.sync.dma_start(out=psid[ia * n2 + ib], in_=psi2[:])

                    h2acc = l2m.tile([C3, 2, HW], f32)
                    for k, (ki, kj) in enumerate(_taps):
                        # two bf16 half-accumulators shorten the serial chain
                        A1 = l2tr.tile([C3, HW], bf16, tag="A1")
                        A2 = l2tr.tile([C3, HW], bf16, tag="A2")
                        na1 = na2 = 0
                        nterm = 0
                        for ia in range(n2):
                            for ib in range(n2):
                                if "l2mod" in _SKIP:
                                    continue
                                psir = l2tr.tile([C3, HW], bf16, tag="psir")
                                for sub in range(2):
                                    nc.sync.dma_start(
                                        out=psir[sub * C2:(sub + 1) * C2, :],
                                        in_=rowbcast(
                                            psid,
                                            ((ia * n2 + ib) * 18 + sub * K2 + k) * HW,
                                            C2, HW))
                                hwa = bass.AP(
                                    tensor=h1p[:].tensor,
                                    offset=h1p[:].offset + pair * (HP2 * WP2)
                                    + (PAD2 + ki + ia - r2) * WP2
                                    + (PAD2 + kj + ib - r2),
                                    ap=[h1p[:].ap[0], [WP2, H], [1, W]])
                                if "l2dve" in _SKIP:
                                    continue
                                P = l2tr.tile([C3, HW], bf16, tag="P2")
                                nc.vector.tensor_mul(out=P[:], in0=psir[:], in1=hwa)
                                if nterm % 2 == 0:
                                    if na1 == 0:
                                        nc.vector.tensor_copy(out=A1[:], in_=P[:])
                                    else:
                                        nc.vector.tensor_add(out=A1[:], in0=A1[:], in1=P[:])
                                    na1 += 1
                                else:
                                    if na2 == 0:
                                        nc.vector.tensor_copy(out=A2[:], in_=P[:])
                                    else:
                                        nc.vector.tensor_add(out=A2[:], in0=A2[:], in1=P[:])
                                    na2 += 1
                                nterm += 1
                        nc.vector.tensor_add(out=A1[:], in0=A1[:], in1=A2[:])
                        for sub in range(2):
                            for chk in range(NCHUNK):
                                ps = ps2b.tile([C3, CH], f32, tag="ps_b")
                                nc.tensor.matmul(
                                    ps[:],
                                    t_wconv2[sub * C2:(sub + 1) * C2, k, :],
                                    A1[sub * C2:(sub + 1) * C2,
                                       chk * CH:(chk + 1) * CH],
                                    start=True, stop=True, tile_position=(sub * C2, 0))
                                dst = h2acc[:, sub, chk * CH:(chk + 1) * CH]
                                if k == 0:
                                    nc.vector.tensor_copy(out=dst, in_=ps[:])
                                else:
                                    nc.vector.tensor_add(out=dst, in0=dst, in1=ps[:])

                    for sub in range(2):
                        img = pair * 2 + sub
                        h2 = l2m.tile([C3, HW], bf16, tag="h2")
                        nc.scalar.activation(out=h2[:], in_=h2acc[:, sub, :],
                                             func=AF.Relu,
                                             bias=t_bn2[:, 1:2], scale=t_bn2[:, 0:1])
                        nc.vector.tensor_reduce(out=pooled[:, img:img + 1], in_=h2[:],
                                                axis=AX.X, op=ALU.add)

        # ============================ FC head ============================
        with tc.tile_pool(name="fc", bufs=1) as fcp, \
             tc.tile_pool(name="psf", bufs=2, space="PSUM") as psf:
            y1 = fcp.tile([C3, 2, BPC], f32)
            for mh in range(2):
                ps = psf.tile([C3, BPC], f32, tag="f1")
                nc.tensor.matmul(ps[:],
                                 t_fc1[:, mh * C3:(mh + 1) * C3],
                                 pooled[:], start=True, stop=True)
                nc.scalar.activation(out=y1[:, mh, :], in_=ps[:], func=AF.Relu,
                                     bias=t_fc1b[:, mh:mh + 1], scale=1.0)
            yout = fcp.tile([100, 2, BPC], f32)
            for mh in range(2):
                ps = psf.tile([100, BPC], f32, tag="f2")
                for kc in range(2):
                    nc.tensor.matmul(ps[:],
                                     t_fc2[:, kc, mh * 100:(mh + 1) * 100],
                                     y1[:, kc, :],
                                     start=(kc == 0), stop=(kc == 1))
                nc.vector.tensor_scalar(out=yout[:, mh, :], in0=ps[:],
                                        scalar1=t_fc2b[:, mh:mh + 1], scalar2=None,
                                        op0=ALU.add)
            for mh in range(2):
                ya = y[:]
                dst = bass.AP(tensor=ya.tensor, offset=ya.offset + mh * 100,
                              ap=[[1, 100], [200, BPC]])
                nc.sync.dma_start(out=dst, in_=yout[:, mh, :])

    return nc


# --------------------------------------------------------------------------
# host prep + entry point
# --------------------------------------------------------------------------

def _host_prep(d, r1, r2):
    import ml_dtypes
    n1, n2 = 2 * r1 + 1, 2 * r2 + 1

    w_off1 = np.zeros((32, 18), np.float32)
    w_conv1 = np.zeros((32, C2), np.float32)
    for k in range(K2):
        ki, kj = k // 3, k % 3
        for c in range(C1):
            w_off1[k * 3 + c, :] = d["offset1_w"][:, c, ki, kj]
            w_conv1[k * 3 + c, :] = d["conv1_w"][:, c, ki, kj]
    w_off1 = np.tile(w_off1, (4, 1))            # replicate per image block
    w_conv1 = np.tile(w_conv1, (4, 1))

    w_off2 = np.zeros((C2, K2, 18), np.float32)
    w_conv2 = np.zeros((C2, K2, C3), np.float32)
    for k in range(K2):
        ki, kj = k // 3, k % 3
        w_off2[:, k, :] = d["offset2_w"][:, :, ki, kj].T
        w_conv2[:, k, :] = d["conv2_w"][:, :, ki, kj].T
    w_off2 = np.tile(w_off2, (2, 1, 1))         # replicate per image sub block
    w_conv2 = np.tile(w_conv2, (2, 1, 1))

    s1 = d["bn1_gamma"] / np.sqrt(d["bn1_var"] + EPS)
    t1 = (d["conv1_b"] - d["bn1_mean"]) * s1 + d["bn1_beta"]
    bn1v = np.stack([np.tile(s1, 2), np.tile(t1, 2)], axis=1)     # [128, 2]
    s2 = d["bn2_gamma"] / np.sqrt(d["bn2_var"] + EPS)
    t2 = (d["conv2_b"] - d["bn2_mean"]) * s2 + d["bn2_beta"]
    bn2v = np.stack([s2, t2], axis=1)                             # [128, 2]

    b1 = d["offset1_b"]
    tyb1 = np.zeros((C3, 2 * n1), np.float32)
    for r in range(C3):
        k = min((r % 32) // 3, K2 - 1)
        for i in range(n1):
            tyb1[r, i] = b1[2 * k] - (i - r1)
            tyb1[r, n1 + i] = b1[2 * k + 1] - (i - r1)
    b2 = d["offset2_b"]
    tyb2 = np.zeros((18, 2 * n2), np.float32)
    for r in range(18):
        k = r % K2
        for i in range(n2):
            tyb2[r, i] = b2[2 * k] - (i - r2)
            tyb2[r, n2 + i] = b2[2 * k + 1] - (i - r2)

    return {
        "w_off1": w_off1, "w_conv1": w_conv1.astype(ml_dtypes.bfloat16),
        "w_off2": w_off2.astype(ml_dtypes.bfloat16),
        "w_conv2": w_conv2.astype(ml_dtypes.bfloat16),
        "bn1v": bn1v, "bn2v": bn2v, "tyb1": tyb1, "tyb2": tyb2,
        "fc1_lT": np.ascontiguousarray((d["fc1_w"] / HW).T),
        "fc1_b": np.ascontiguousarray(d["fc1_b"].reshape(2, C3).T),
        "fc2_lT": np.ascontiguousarray(
            d["fc2_w"].T.reshape(2, C3, 200).transpose(1, 0, 2)),
        "fc2_b": np.ascontiguousarray(d["fc2_b"].reshape(2, 100).T),
    }


def kernel(**inputs):
    global _LAST_RESULTS
    from concourse.bass_utils import run_bass_kernel_spmd

    d = {k: np.asarray(v, np.float32) for k, v in inputs.items()}
    r1, r2, live1 = _window_radii(d)
    assert r1 <= 2 and r2 <= 2, (r1, r2)

    base = _host_prep(d, r1, r2)
    nc = _build_program(r1, r2, live1)

    x = d["x"]
    in_maps = []
    for core in range(NCORES):
        m = dict(base)
        m["x4"] = np.ascontiguousarray(x[core * BPC:(core + 1) * BPC])
        in_maps.append(m)
    res = run_bass_kernel_spmd(nc, in_maps, core_ids=list(range(NCORES)),
                               trace=_TRACE)
    _LAST_RESULTS = res
    return np.concatenate([r["y"] for r in res.results], axis=0).astype(np.float32)
